# revision 8
# baseline (speedup 1.0000x reference)
"""Trainium2 Bass kernel for nn_LELoss (retrieval_knn).

Math (per cloud b of B=32, N=16384 points, L=128 landmarks):
    dists[l,n] = ||pre_lm[l] - pre_xyz[n]||^2
    top-3 smallest per landmark -> inverse-distance weights ->
    lm_dis[l] = sum_k w_k * pred_dis[idx_k];  pred_lm = pre_lm + lm_dis
    loss = mean over (B,L) of sum_c (target_lm - pred_lm)^2

Device strategy (data-parallel, 4 clouds per core x 8 cores):
    score s'[l,n] = -dists[l,n] computed EXACTLY as one K=5 matmul:
        lhsT rows: [2*plx, 2*ply, 2*plz, 1, -|pl|^2]
        rhs  rows: [x, y, z, -|x|^2, 1]
    (float32r matmul: 1 col/cycle, fp32-class precision)
    -> PSUM -> ACT evict to SBUF fp32 -> DVE max8 (top-8 values, descending)
    -> DVE max_index (their indices) -> DMA out.
Host does the (0.1% of FLOPs) epilogue: gather pred_dis at top-3 indices,
weights, pred_lm, loss.
"""

import os
import numpy as np

B = 32
N = 16384
L = 128
NCORES = 8
CPC = B // NCORES          # clouds per core
CHUNK = 2048               # rhs stream chunk (free dim)
NCHUNK = N // CHUNK
MMFREE = 512               # matmul moving free dim (one PSUM bank)

_CACHE = {}
LAST_RESULTS = None        # test.py introspection


def _build_module():
    import concourse.bass as bass  # noqa: F401
    import concourse.tile as tile
    from concourse import bacc, mybir
    from contextlib import ExitStack

    nc = bacc.Bacc(
        "TRN2",
        target_bir_lowering=False,
        debug=False,
        enable_asserts=False,
        num_devices=NCORES,
    )
    f32 = mybir.dt.float32
    f32r = mybir.dt.float32r
    u32 = mybir.dt.uint32

    rhs_d = nc.dram_tensor("rhs", [CPC, 8, N], f32, kind="ExternalInput").ap()
    lhsT_d = nc.dram_tensor("lhsT", [CPC, 8, L], f32, kind="ExternalInput").ap()
    idxs_d = nc.dram_tensor("idxs", [CPC, L, 8], u32, kind="ExternalOutput").ap()

    with tile.TileContext(nc) as tc, ExitStack() as ctx:
        lhs_pool = ctx.enter_context(tc.tile_pool(name="lhs", bufs=2))
        rhs_pool = ctx.enter_context(tc.tile_pool(name="rhsp", bufs=3))
        s_pool = ctx.enter_context(tc.tile_pool(name="scores", bufs=2))
        tree_pool = ctx.enter_context(tc.tile_pool(name="tree", bufs=2))
        psum_pool = ctx.enter_context(tc.tile_pool(name="ps", bufs=2, space="PSUM"))
        o_pool = ctx.enter_context(tc.tile_pool(name="outs", bufs=4))

        bf16 = mybir.dt.bfloat16
        for i in range(CPC):
            lt = lhs_pool.tile([8, L], f32)
            nc.sync.dma_start(lt[:], lhsT_d[i])
            S = s_pool.tile([128, N], bf16)
            for j in range(NCHUNK):
                rt = rhs_pool.tile([8, CHUNK], f32)
                nc.sync.dma_start(rt[:], rhs_d[i, :, j * CHUNK:(j + 1) * CHUNK])
                ps = psum_pool.tile([128, CHUNK], f32)
                for k in range(CHUNK // MMFREE):
                    nc.tensor.matmul(
                        ps[:, k * MMFREE:(k + 1) * MMFREE],
                        lt[0:5, :],
                        rt[0:5, k * MMFREE:(k + 1) * MMFREE],
                        start=True,
                        stop=True,
                    )
                nc.scalar.copy(S[:, j * CHUNK:(j + 1) * CHUNK], ps[:])
            # 3-level pairwise max tree: Mg[g] = max_j S[g + j*2048], j=0..7
            t1 = tree_pool.tile([128, N // 2], bf16)
            nc.vector.tensor_max(t1[:], S[:, :N // 2], S[:, N // 2:])
            t2 = tree_pool.tile([128, N // 4], bf16, tag="t2")
            nc.vector.tensor_max(t2[:], t1[:, :N // 4], t1[:, N // 4:])
            mg = tree_pool.tile([128, N // 8], bf16, tag="mg")
            nc.vector.tensor_max(mg[:], t2[:, :N // 8], t2[:, N // 8:])
            vals = o_pool.tile([128, 8], bf16)
            nc.vector.max(vals[:], mg[:])
            idxs = o_pool.tile([128, 8], u32)
            nc.vector.max_index(idxs[:], vals[:], mg[:])
            nc.sync.dma_start(idxs_d[i], idxs[:])

    nc.compile()
    return nc


def _get_module():
    if "nc" not in _CACHE:
        _CACHE["nc"] = _build_module()
    return _CACHE["nc"]


def kernel(pred_dis, pre_xyz, pre_lm, target_lm, batch):
    global LAST_RESULTS
    from concourse import bass_utils

    pred_dis = np.asarray(pred_dis, dtype=np.float32)
    pre_xyz = np.asarray(pre_xyz, dtype=np.float32)
    pre_lm = np.asarray(pre_lm, dtype=np.float32)
    target_lm = np.asarray(target_lm, dtype=np.float32)

    px = pre_xyz.reshape(B, N, 3)
    pl = pre_lm.reshape(B, L, 3)

    # host shard prep: transposed coordinate rows + folded norm rows
    rhs = np.zeros((B, 8, N), dtype=np.float32)
    rhs[:, 0:3] = px.transpose(0, 2, 1)
    rhs[:, 3] = -np.square(px).sum(-1)
    rhs[:, 4] = 1.0
    lhsT = np.zeros((B, 8, L), dtype=np.float32)
    lhsT[:, 0:3] = 2.0 * pl.transpose(0, 2, 1)
    lhsT[:, 3] = 1.0
    lhsT[:, 4] = -np.square(pl).sum(-1)

    in_maps = []
    for c in range(NCORES):
        sl = slice(c * CPC, (c + 1) * CPC)
        in_maps.append({
            "rhs": np.ascontiguousarray(rhs[sl]),
            "lhsT": np.ascontiguousarray(lhsT[sl]),
        })

    nc = _get_module()
    res = bass_utils.run_bass_kernel_spmd(
        nc, in_maps, core_ids=list(range(NCORES)),
        trace=bool(os.environ.get("BASS_TRACE")),
    )
    LAST_RESULTS = res

    idxs = np.concatenate([r["idxs"] for r in res.results], axis=0)  # (B,L,8)

    # Device returns the top-8 GROUPS per landmark (group g = points
    # {g + j*2048, j=0..7}, the pairwise-max tree's strided grouping).
    # The top-3 points provably live in the top-3 groups by group-max;
    # top-8 gives margin against bf16/dot-form rounding. Re-rank all 64
    # member candidates with the exact subtract-first form, take top-3.
    gi = idxs.astype(np.int64)                                # (B,L,8)
    cand = (gi[..., :, None] + np.arange(8, dtype=np.int64)[None, None, None, :]
            * (N // 8)).reshape(B, L, 64)
    cand_xyz = np.take_along_axis(px[:, None, :, :],
                                  cand[..., None], axis=2)    # (B,L,64,3)
    d2c = np.square(cand_xyz - pl[:, :, None, :]).sum(-1)     # (B,L,64) fp32
    ord3 = np.argsort(d2c, axis=2, kind="stable")[:, :, :3]
    top3i = np.take_along_axis(cand, ord3, axis=2)
    d2 = np.take_along_axis(d2c, ord3, axis=2)

    kd = 1.0 / (d2 + 1e-8)
    w = kd / kd.sum(-1, keepdims=True)

    pd = pred_dis.reshape(B, N, 3)
    gathered = np.take_along_axis(pd[:, None, :, :], top3i[..., None], axis=2)
    lm_dis = (gathered * w[..., None]).sum(2)    # (B,L,3)
    pred_lm = pl + lm_dis
    tl = target_lm.reshape(B, L, 3)
    loss = np.mean(np.sum((tl - pred_lm) ** 2, axis=-1), dtype=np.float32)
    return np.float32(loss), pred_lm.reshape(-1, 3).astype(np.float32)
